# revision 34
# baseline (speedup 1.0000x reference)
"""Per-sample batched matmul: out[b,o,f] = sum_i weights[b,o,i] * x[b,i,f].

Sharding: batch (bs=32) split across 8 NeuronCores, 4 samples each, zero
communication.

Full-bf16 datapath, software-pipelined across engines:
- Both matmul operands are cast to bf16 on-chip (walrus forbids 32/16-bit
  mixing), which enables the PE's fast-weight-load path: LDWEIGHTS drops
  to ~97ns and hides under the 512-cycle moving stream, pacing matmuls at
  ~216ns vs f32r's ~227ns. Accumulation stays fp32 in PSUM; measured
  rel err ~3e-3 vs the 2e-2 gate.
- W pipeline per sample: DMA (sync ring) -> DVE/ACT cast to bf16 -> 8 PE
  transposes into one PSUM bank -> one wide eviction into the [I,O]
  stationary layout. Sample b+1's stations are emitted between sample
  b's matmul groups so per-engine FIFO order never stalls the PE at a
  sample boundary; each W DMA goes one group before its station (big
  up-front W bursts carry buffer-reuse waits that convoy the HWDGE ring).
- x chunks (1MB): chunks 0-1 ride the gpsimd/SWDGE ring (idle at
  startup, in parallel with the sync ring's W blocks), later chunks ride
  sync, issued 3 ahead; the f32->bf16 cast splits across DVE+ACT and is
  emitted mid-chunk so it never head-of-line blocks eviction copies.
- device output in bf16 (host upcasts): halves output HBM traffic so
  input streams keep the ~358 GB/s per-core HBM budget, and shortens the
  drain tail; outputs ride GpSimd/SWDGE except the last chunk, which
  drains on the by-then-idle sync ring (shorter end-of-kernel barrier).
"""

import sys

try:  # concourse (Bass/Tile) ships in the container, not on default sys.path
    import concourse  # noqa: F401
except ImportError:
    sys.path.insert(0, "/opt/trn_rl_repo")

import numpy as np

BS, IN_SIZE, OUT_SIZE, FEATS = 32, 1024, 1024, 2048
N_CORES = 8
BPC = BS // N_CORES  # samples per core

P = 128
N_FREE = 512  # moving-operand free dim per matmul (1 PSUM bank of fp32)
KO = IN_SIZE // P  # 8 contraction tiles
MO = OUT_SIZE // P  # 8 output-row tiles
NF = FEATS // N_FREE  # 4 output-col chunks
NCHUNK = BPC * NF  # 16 x-chunks, processed in order

_NC_CACHE = {}


def _build_nc():
    import concourse.mybir as mybir
    import concourse.tile as tile
    from concourse import bacc

    f32 = mybir.dt.float32
    f32r = mybir.dt.float32r
    bf16 = mybir.dt.bfloat16

    import ml_dtypes

    nc = bacc.Bacc("TRN2", target_bir_lowering=False, debug=False)
    x_d = nc.dram_tensor(
        "x", [BPC, IN_SIZE, FEATS], f32, kind="ExternalInput"
    ).ap()
    w_d = nc.dram_tensor(
        "w", [BPC, OUT_SIZE, IN_SIZE], f32, kind="ExternalInput"
    ).ap()
    o_d = nc.dram_tensor(
        "out", [BPC, OUT_SIZE, FEATS], bf16, kind="ExternalOutput"
    ).ap()

    with tile.TileContext(nc) as tc:
        with (
            tc.tile_pool(name="const", bufs=1) as const,
            tc.tile_pool(name="wn_pool", bufs=12) as wn_pool,
            tc.tile_pool(name="wnb_pool", bufs=3) as wnb_pool,
            tc.tile_pool(name="wt_pool", bufs=2) as wt_pool,
            tc.tile_pool(name="xn_pool", bufs=4) as xn_pool,
            tc.tile_pool(name="xnb_pool", bufs=4) as xnb_pool,
            tc.tile_pool(name="ot_pool", bufs=12) as ot_pool,
            tc.tile_pool(name="psmm", bufs=6, space="PSUM") as psmm_pool,
            tc.tile_pool(name="pstr", bufs=2, space="PSUM") as pstr_pool,
        ):
            eye_d = nc.inline_tensor(
                np.eye(P, dtype=ml_dtypes.bfloat16), name="eye"
            )
            ident = const.tile([P, P], bf16, name="identr")
            nc.sync.dma_start(ident[:], eye_d.ap())

            # alternate DVE/ACT for every eviction so neither engine's
            # FIFO becomes the critical path
            par = {"i": 0}

            def alt_copy(dst, src):
                par["i"] += 1
                if par["i"] % 2 == 0:
                    nc.vector.tensor_copy(out=dst, in_=src)
                else:
                    nc.scalar.copy(dst, src)

            xr = [x_d[b].rearrange("(ko p) f -> p ko f", p=P) for b in range(BPC)]
            xn_f = {}  # chunk -> f32 staging tile
            xn = {}  # chunk -> bf16 x tile
            wn = {}  # (b, mo) -> f32r W row-block
            wt = {}  # b -> [P, KO, MO, P] f32r stationary layout

            def issue_xdma(k):
                b, n = divmod(k, NF)
                t = xn_pool.tile([P, KO, N_FREE], f32, tag="xn", name=f"xn_{k}")
                nc.sync.dma_start(
                    t[:], xr[b][:, :, n * N_FREE : (n + 1) * N_FREE]
                )
                xn_f[k] = t

            def emit_xcast(k):
                t = xnb_pool.tile(
                    [P, KO, N_FREE], bf16, tag="xnb", name=f"xnb_{k}"
                )
                h = KO // 2
                src_t = xn_f.pop(k)
                nc.vector.tensor_copy(out=t[:, :h], in_=src_t[:, :h])
                nc.scalar.copy(t[:, h:], src_t[:, h:])
                xn[k] = t

            def issue_wdma(b, mo, ways=1):
                t = wn_pool.tile([P, IN_SIZE], f32, tag="wn", name=f"wn_{b}_{mo}")
                src = w_d[b, mo * P : (mo + 1) * P, :]
                w = IN_SIZE // ways
                for q in range(ways):
                    nc.sync.dma_start(
                        t[:, q * w : (q + 1) * w], src[:, q * w : (q + 1) * w]
                    )
                wn[(b, mo)] = t

            def emit_w_station(b, mo):
                """cast one W row-block to bf16, transpose its 8 tiles into
                one PSUM bank, leave via one wide copy (DVE/ACT alternating)."""
                wb = wnb_pool.tile(
                    [P, IN_SIZE], bf16, tag="wnb", name=f"wnb_{b}_{mo}"
                )
                alt_copy(wb[:], wn.pop((b, mo))[:])
                pt = pstr_pool.tile([P, KO * P], bf16, tag="pt", name=f"pt_{b}_{mo}")
                for ko in range(KO):
                    nc.tensor.transpose(
                        pt[:, ko * P : (ko + 1) * P],
                        wb[:, ko * P : (ko + 1) * P],
                        ident[:],
                    )
                alt_copy(
                    wt[b][:, :, mo, :],
                    pt[:].rearrange("p (k q) -> p k q", k=KO),
                )

            def mm_group(k, mo):
                """One [128, 512] output tile: 8 accumulating matmuls, a
                cast-evict to bf16, and an output DMA on GpSimd (SWDGE) so
                compute-lagged output waits never block input prefetch. The
                final groups instead split the evict across DVE+ACT and
                drain on the (idle by then) sync ring for a shorter tail."""
                b, n = divmod(k, NF)
                xt = xn[k]
                ps = psmm_pool.tile([P, N_FREE], f32, tag="ps", name=f"ps_{k}_{mo}")
                for ko in range(KO):
                    nc.tensor.matmul(
                        ps[:],
                        wt[b][:, ko, mo, :],
                        xt[:, ko, :],
                        start=(ko == 0),
                        stop=(ko == KO - 1),
                    )
                ot = ot_pool.tile([P, N_FREE], bf16, tag="ot", name=f"ot_{k}_{mo}")
                dst = o_d[b, mo * P : (mo + 1) * P, n * N_FREE : (n + 1) * N_FREE]
                if k == NCHUNK - 1 and mo == MO - 1:
                    h = N_FREE // 2
                    nc.vector.tensor_copy(out=ot[:, :h], in_=ps[:, :h])
                    nc.scalar.copy(ot[:, h:], ps[:, h:])
                    nc.sync.dma_start(dst[:, :h], ot[:, :h])
                    nc.sync.dma_start(dst[:, h:], ot[:, h:])
                    return
                alt_copy(ot[:], ps[:])
                if k == NCHUNK - 1:
                    # whole last chunk drains on sync: the gpsimd queue's
                    # end-of-kernel drain barrier then has nothing left to
                    # wait for (SWDGE completion latency ~1us vs 0.6)
                    nc.sync.dma_start(dst, ot[:])
                else:
                    nc.gpsimd.dma_start(dst, ot[:])

            # ---- HAM warmup: ~3.4us of identity transposes while the first
            # DMAs are in flight, so the real work starts on a warm PE.
            warm_sink = const.tile([P, 16], bf16, name="warm_sink")
            for wg in range(6):
                ptw = pstr_pool.tile([P, KO * P], bf16, tag="pt", name=f"ptw_{wg}")
                for c in range(KO):
                    nc.tensor.transpose(
                        ptw[:, c * P : (c + 1) * P], ident[:], ident[:]
                    )
                nc.vector.tensor_copy(out=warm_sink[:], in_=ptw[:, :16])

            for b in range(BPC):
                wt[b] = wt_pool.tile(
                    [P, KO, MO, P], bf16, tag="wt", name=f"wt_{b}"
                )

            # ---- startup: sample 0's W pipeline interleaves with its first
            # chunk's matmul groups, paced by the arriving DMAs.
            # chunk 0's DMA lands as two halves so its bf16 cast can start
            # as soon as the first half arrives
            t0x = xn_pool.tile([P, KO, N_FREE], f32, tag="xn", name="xn_0")
            h = KO // 2
            nc.gpsimd.dma_start(t0x[:, :h], xr[0][:, :h, 0:N_FREE])
            nc.gpsimd.dma_start(t0x[:, h:], xr[0][:, h:, 0:N_FREE])
            xn_f[0] = t0x
            t1x = xn_pool.tile([P, KO, N_FREE], f32, tag="xn", name="xn_1")
            nc.gpsimd.dma_start(t1x[:], xr[0][:, :, N_FREE : 2 * N_FREE])
            xn_f[1] = t1x
            issue_wdma(0, 0, ways=2)
            issue_wdma(0, 1, ways=2)
            issue_wdma(0, 2, ways=2)
            issue_wdma(0, 3, ways=2)
            for mo in range(4, MO):
                issue_wdma(0, mo)
            emit_w_station(0, 0)
            t0b = xnb_pool.tile([P, KO, N_FREE], bf16, tag="xnb", name="xnb_0")
            q = KO // 4
            for i in range(4):
                sl = slice(i * q, (i + 1) * q)
                if i % 2 == 0:
                    nc.vector.tensor_copy(out=t0b[:, sl], in_=t0x[:, sl])
                else:
                    nc.scalar.copy(t0b[:, sl], t0x[:, sl])
            xn_f.pop(0)
            xn[0] = t0b
            for mo in range(MO):
                mm_group(0, mo)
                if mo + 1 < MO:
                    emit_w_station(0, mo + 1)
                if mo == 1:
                    issue_xdma(2)
                if mo == 2:
                    emit_xcast(1)
                if mo == 3:
                    issue_xdma(3)

            # ---- steady state: chunk k runs its 8 groups; meanwhile chunk
            # k+3's DMA is issued, sample b+1's W DMAs are issued during
            # local chunks n=0,1, and its W stations (transpose + evict)
            # are emitted between groups during n=1,2.
            for k in range(1, NCHUNK):
                b, n = divmod(k, NF)
                if k + 3 < NCHUNK:
                    issue_xdma(k + 3)
                for mo in range(MO):
                    mm_group(k, mo)
                    if mo == 3 and k + 1 < NCHUNK:
                        emit_xcast(k + 1)
                    if n in (1, 2) and b + 1 < BPC:
                        j = (n - 1) * (MO // 2) + mo // 2
                        if mo % 2 == 0:
                            issue_wdma(b + 1, j)
                        else:
                            emit_w_station(b + 1, j)

    nc.compile()
    return nc


def run(x, weights, trace=False):
    """Shard on batch, run SPMD on 8 cores, gather. Returns (out, results)."""
    from concourse.bass_utils import run_bass_kernel_spmd

    key = "nc"
    if key not in _NC_CACHE:
        _NC_CACHE[key] = _build_nc()
    nc = _NC_CACHE[key]

    x = np.ascontiguousarray(np.asarray(x, dtype=np.float32))
    weights = np.ascontiguousarray(np.asarray(weights, dtype=np.float32))
    in_maps = [
        {
            "x": x[c * BPC : (c + 1) * BPC],
            "w": weights[c * BPC : (c + 1) * BPC],
        }
        for c in range(N_CORES)
    ]
    last_err = None
    for attempt in range(5):
        try:
            res = run_bass_kernel_spmd(
                nc, in_maps, core_ids=list(range(N_CORES)), trace=trace
            )
            break
        except Exception as e:  # transient NRT device faults: back off, retry
            last_err = e
            import time as _time

            _time.sleep(10 * (attempt + 1))
    else:
        raise last_err
    out = np.concatenate(
        [
            np.asarray(res.results[c]["out"]).astype(np.float32)
            for c in range(N_CORES)
        ],
        axis=0,
    )
    return out, res


def kernel(x, weights):
    out, _ = run(x, weights, trace=False)
    return out


# revision 35
# speedup vs baseline: 1.0206x; 1.0206x over previous
"""Per-sample batched matmul: out[b,o,f] = sum_i weights[b,o,i] * x[b,i,f].

Sharding: batch (bs=32) split across 8 NeuronCores, 4 samples each, zero
communication.

Full-bf16 datapath, software-pipelined across engines:
- Both matmul operands are cast to bf16 on-chip (walrus forbids 32/16-bit
  mixing), which enables the PE's fast-weight-load path: LDWEIGHTS drops
  to ~97ns and hides under the 512-cycle moving stream, pacing matmuls at
  ~216ns vs f32r's ~227ns. Accumulation stays fp32 in PSUM; measured
  rel err ~3e-3 vs the 2e-2 gate.
- W pipeline per sample: DMA (sync ring) -> DVE/ACT cast to bf16 -> 8 PE
  transposes into one PSUM bank -> one wide eviction into the [I,O]
  stationary layout. Sample b+1's stations are emitted between sample
  b's matmul groups so per-engine FIFO order never stalls the PE at a
  sample boundary; each W DMA goes one group before its station (big
  up-front W bursts carry buffer-reuse waits that convoy the HWDGE ring).
- x chunks (1MB): chunks 0-1 ride the gpsimd/SWDGE ring (idle at
  startup, in parallel with the sync ring's W blocks), later chunks ride
  sync, issued 3 ahead; the f32->bf16 cast splits across DVE+ACT and is
  emitted mid-chunk so it never head-of-line blocks eviction copies.
- device output in bf16 (host upcasts): halves output HBM traffic so
  input streams keep the ~358 GB/s per-core HBM budget, and shortens the
  drain tail; outputs ride GpSimd/SWDGE except the last chunk, which
  drains on the by-then-idle sync ring (shorter end-of-kernel barrier).
"""

import sys

try:  # concourse (Bass/Tile) ships in the container, not on default sys.path
    import concourse  # noqa: F401
except ImportError:
    sys.path.insert(0, "/opt/trn_rl_repo")

import numpy as np

BS, IN_SIZE, OUT_SIZE, FEATS = 32, 1024, 1024, 2048
N_CORES = 8
BPC = BS // N_CORES  # samples per core

P = 128
N_FREE = 512  # moving-operand free dim per matmul (1 PSUM bank of fp32)
KO = IN_SIZE // P  # 8 contraction tiles
MO = OUT_SIZE // P  # 8 output-row tiles
NF = FEATS // N_FREE  # 4 output-col chunks
NCHUNK = BPC * NF  # 16 x-chunks, processed in order

_NC_CACHE = {}


def _build_nc():
    import concourse.mybir as mybir
    import concourse.tile as tile
    from concourse import bacc

    f32 = mybir.dt.float32
    f32r = mybir.dt.float32r
    bf16 = mybir.dt.bfloat16

    import ml_dtypes

    nc = bacc.Bacc("TRN2", target_bir_lowering=False, debug=False)
    x_d = nc.dram_tensor(
        "x", [BPC, IN_SIZE, FEATS], f32, kind="ExternalInput"
    ).ap()
    w_d = nc.dram_tensor(
        "w", [BPC, OUT_SIZE, IN_SIZE], f32, kind="ExternalInput"
    ).ap()
    o_d = nc.dram_tensor(
        "out", [BPC, OUT_SIZE, FEATS], bf16, kind="ExternalOutput"
    ).ap()

    with tile.TileContext(nc) as tc:
        with (
            tc.tile_pool(name="const", bufs=1) as const,
            tc.tile_pool(name="wn_pool", bufs=10) as wn_pool,
            tc.tile_pool(name="wnb_pool", bufs=3) as wnb_pool,
            tc.tile_pool(name="wt_pool", bufs=2) as wt_pool,
            tc.tile_pool(name="xn_pool", bufs=5) as xn_pool,
            tc.tile_pool(name="xnb_pool", bufs=4) as xnb_pool,
            tc.tile_pool(name="ot_pool", bufs=10) as ot_pool,
            tc.tile_pool(name="psmm", bufs=6, space="PSUM") as psmm_pool,
            tc.tile_pool(name="pstr", bufs=2, space="PSUM") as pstr_pool,
        ):
            eye_d = nc.inline_tensor(
                np.eye(P, dtype=ml_dtypes.bfloat16), name="eye"
            )
            ident = const.tile([P, P], bf16, name="identr")
            nc.sync.dma_start(ident[:], eye_d.ap())

            # alternate DVE/ACT for every eviction so neither engine's
            # FIFO becomes the critical path
            par = {"i": 0}

            def alt_copy(dst, src):
                par["i"] += 1
                if par["i"] % 2 == 0:
                    nc.vector.tensor_copy(out=dst, in_=src)
                else:
                    nc.scalar.copy(dst, src)

            xr = [x_d[b].rearrange("(ko p) f -> p ko f", p=P) for b in range(BPC)]
            xn_f = {}  # chunk -> f32 staging tile
            xn = {}  # chunk -> bf16 x tile
            wn = {}  # (b, mo) -> f32r W row-block
            wt = {}  # b -> [P, KO, MO, P] f32r stationary layout

            def issue_xdma(k):
                b, n = divmod(k, NF)
                t = xn_pool.tile([P, KO, N_FREE], f32, tag="xn", name=f"xn_{k}")
                nc.sync.dma_start(
                    t[:], xr[b][:, :, n * N_FREE : (n + 1) * N_FREE]
                )
                xn_f[k] = t

            def emit_xcast(k):
                t = xnb_pool.tile(
                    [P, KO, N_FREE], bf16, tag="xnb", name=f"xnb_{k}"
                )
                h = KO // 2
                src_t = xn_f.pop(k)
                nc.vector.tensor_copy(out=t[:, :h], in_=src_t[:, :h])
                nc.scalar.copy(t[:, h:], src_t[:, h:])
                xn[k] = t

            def issue_wdma(b, mo, ways=1):
                t = wn_pool.tile([P, IN_SIZE], f32, tag="wn", name=f"wn_{b}_{mo}")
                src = w_d[b, mo * P : (mo + 1) * P, :]
                w = IN_SIZE // ways
                for q in range(ways):
                    nc.sync.dma_start(
                        t[:, q * w : (q + 1) * w], src[:, q * w : (q + 1) * w]
                    )
                wn[(b, mo)] = t

            def emit_w_station(b, mo):
                """cast one W row-block to bf16, transpose its 8 tiles into
                one PSUM bank, leave via one wide copy (DVE/ACT alternating)."""
                wb = wnb_pool.tile(
                    [P, IN_SIZE], bf16, tag="wnb", name=f"wnb_{b}_{mo}"
                )
                alt_copy(wb[:], wn.pop((b, mo))[:])
                pt = pstr_pool.tile([P, KO * P], bf16, tag="pt", name=f"pt_{b}_{mo}")
                for ko in range(KO):
                    nc.tensor.transpose(
                        pt[:, ko * P : (ko + 1) * P],
                        wb[:, ko * P : (ko + 1) * P],
                        ident[:],
                    )
                alt_copy(
                    wt[b][:, :, mo, :],
                    pt[:].rearrange("p (k q) -> p k q", k=KO),
                )

            def mm_group(k, mo):
                """One [128, 512] output tile: 8 accumulating matmuls, a
                cast-evict to bf16, and an output DMA on GpSimd (SWDGE) so
                compute-lagged output waits never block input prefetch. The
                final groups instead split the evict across DVE+ACT and
                drain on the (idle by then) sync ring for a shorter tail."""
                b, n = divmod(k, NF)
                xt = xn[k]
                ps = psmm_pool.tile([P, N_FREE], f32, tag="ps", name=f"ps_{k}_{mo}")
                for ko in range(KO):
                    nc.tensor.matmul(
                        ps[:],
                        wt[b][:, ko, mo, :],
                        xt[:, ko, :],
                        start=(ko == 0),
                        stop=(ko == KO - 1),
                    )
                ot = ot_pool.tile([P, N_FREE], bf16, tag="ot", name=f"ot_{k}_{mo}")
                dst = o_d[b, mo * P : (mo + 1) * P, n * N_FREE : (n + 1) * N_FREE]
                if k == NCHUNK - 1 and mo == MO - 1:
                    h = N_FREE // 2
                    nc.vector.tensor_copy(out=ot[:, :h], in_=ps[:, :h])
                    nc.scalar.copy(ot[:, h:], ps[:, h:])
                    nc.sync.dma_start(dst[:, :h], ot[:, :h])
                    nc.sync.dma_start(dst[:, h:], ot[:, h:])
                    return
                alt_copy(ot[:], ps[:])
                if k == NCHUNK - 1:
                    # whole last chunk drains on sync: the gpsimd queue's
                    # end-of-kernel drain barrier then has nothing left to
                    # wait for (SWDGE completion latency ~1us vs 0.6)
                    nc.sync.dma_start(dst, ot[:])
                else:
                    nc.gpsimd.dma_start(dst, ot[:])

            # ---- HAM warmup: ~3.4us of identity transposes while the first
            # DMAs are in flight, so the real work starts on a warm PE.
            warm_sink = const.tile([P, 16], bf16, name="warm_sink")
            junk = const.tile([P, P], bf16, name="junk")
            nc.gpsimd.memset(junk[:], 0.0)
            for wg in range(6):
                ptw = pstr_pool.tile([P, KO * P], bf16, tag="pt", name=f"ptw_{wg}")
                for c in range(KO):
                    nc.tensor.transpose(
                        ptw[:, c * P : (c + 1) * P], junk[:], junk[:]
                    )
                nc.vector.tensor_copy(out=warm_sink[:], in_=ptw[:, :16])

            for b in range(BPC):
                wt[b] = wt_pool.tile(
                    [P, KO, MO, P], bf16, tag="wt", name=f"wt_{b}"
                )

            # ---- startup: sample 0's W pipeline interleaves with its first
            # chunk's matmul groups, paced by the arriving DMAs.
            # chunk 0's DMA lands as two halves so its bf16 cast can start
            # as soon as the first half arrives
            t0x = xn_pool.tile([P, KO, N_FREE], f32, tag="xn", name="xn_0")
            h = KO // 2
            nc.gpsimd.dma_start(t0x[:, :h], xr[0][:, :h, 0:N_FREE])
            nc.gpsimd.dma_start(t0x[:, h:], xr[0][:, h:, 0:N_FREE])
            xn_f[0] = t0x
            t1x = xn_pool.tile([P, KO, N_FREE], f32, tag="xn", name="xn_1")
            nc.gpsimd.dma_start(t1x[:], xr[0][:, :, N_FREE : 2 * N_FREE])
            xn_f[1] = t1x
            issue_wdma(0, 0, ways=2)
            issue_wdma(0, 1, ways=2)
            issue_wdma(0, 2, ways=2)
            issue_wdma(0, 3, ways=2)
            for mo in range(4, MO):
                issue_wdma(0, mo)
            emit_w_station(0, 0)
            t0b = xnb_pool.tile([P, KO, N_FREE], bf16, tag="xnb", name="xnb_0")
            q = KO // 4
            for i in range(4):
                sl = slice(i * q, (i + 1) * q)
                if i % 2 == 0:
                    nc.vector.tensor_copy(out=t0b[:, sl], in_=t0x[:, sl])
                else:
                    nc.scalar.copy(t0b[:, sl], t0x[:, sl])
            xn_f.pop(0)
            xn[0] = t0b
            for mo in range(MO):
                mm_group(0, mo)
                if mo + 1 < MO:
                    emit_w_station(0, mo + 1)
                if mo == 1:
                    issue_xdma(2)
                if mo == 2:
                    emit_xcast(1)
                if mo == 3:
                    issue_xdma(3)

            # ---- steady state: chunk k runs its 8 groups; meanwhile chunk
            # k+3's DMA is issued, sample b+1's W DMAs are issued during
            # local chunks n=0,1, and its W stations (transpose + evict)
            # are emitted between groups during n=1,2.
            for k in range(1, NCHUNK):
                b, n = divmod(k, NF)
                if k + 3 < NCHUNK:
                    issue_xdma(k + 3)
                for mo in range(MO):
                    mm_group(k, mo)
                    if mo == 3 and k + 1 < NCHUNK:
                        emit_xcast(k + 1)
                    if n in (1, 2) and b + 1 < BPC:
                        j = (n - 1) * (MO // 2) + mo // 2
                        if mo % 2 == 0:
                            issue_wdma(b + 1, j)
                        else:
                            emit_w_station(b + 1, j)

    nc.compile()
    return nc


def run(x, weights, trace=False):
    """Shard on batch, run SPMD on 8 cores, gather. Returns (out, results)."""
    from concourse.bass_utils import run_bass_kernel_spmd

    key = "nc"
    if key not in _NC_CACHE:
        _NC_CACHE[key] = _build_nc()
    nc = _NC_CACHE[key]

    x = np.ascontiguousarray(np.asarray(x, dtype=np.float32))
    weights = np.ascontiguousarray(np.asarray(weights, dtype=np.float32))
    in_maps = [
        {
            "x": x[c * BPC : (c + 1) * BPC],
            "w": weights[c * BPC : (c + 1) * BPC],
        }
        for c in range(N_CORES)
    ]
    last_err = None
    for attempt in range(5):
        try:
            res = run_bass_kernel_spmd(
                nc, in_maps, core_ids=list(range(N_CORES)), trace=trace
            )
            break
        except Exception as e:  # transient NRT device faults: back off, retry
            last_err = e
            import time as _time

            _time.sleep(10 * (attempt + 1))
    else:
        raise last_err
    out = np.concatenate(
        [
            np.asarray(res.results[c]["out"]).astype(np.float32)
            for c in range(N_CORES)
        ],
        axis=0,
    )
    return out, res


def kernel(x, weights):
    out, _ = run(x, weights, trace=False)
    return out
